# revision 30
# baseline (speedup 1.0000x reference)
"""DotGAT layer (segment-softmax GNN message passing) on 8 Trainium2 cores, v4.

Strategy (graph/data parallel, custom node relabeling):
  - Nodes are RELABELED host-side: all 50176 (padded) node ids are bin-packed
    into 1568 bins of exactly 32 nodes with near-equal edge counts (~510), via
    degree-sorted snake dealing + local repair.  Bin beta -> core beta//196,
    block (beta%196)//4, segment beta%4.  This kills the max-over-cores chunk
    padding: every (block,segment) needs almost exactly ceil(510/128)=4 chunks
    on every core.  The output rows are scattered back to global order on the
    host.
  - Each 128-edge chunk belongs to ONE 32-node segment, so its destinations
    fall in a fixed SPMD-shared 32-column window: the logit matmul streams 32
    qk columns, exp runs once per segment over the packed [128, Cc*32] PSUM
    tile, the one-hot dst mask is built by ONE broadcast is_equal per segment
    and applied POST-exp as a multiply.
  - Logits use the fused form e = z_src . (B z_dst), B = tau Wk Wq^T.
  - Aggregation: U += zet^T P per chunk (128-wide stationary, FWL-fast).  The
    softmax denominator is ONE matmul per segment (lhsT = the packed P group
    tile, rhs = ones) giving per-chunk partial sums at partitions 32k+i,
    folded onto node rows at block close by 4 constant one-hot matmuls.
  - Slot emission is software-pipelined (SKEW=3): the PE never waits on the
    exp/mask path of the current slot.

The program is recompiled per call with all data-dependent sizes baked in
(SPMD: one instruction stream, 8 cores).
"""

import sys

sys.path.insert(0, "/opt/trn_rl_repo")

import numpy as np

N_NODES = 50000
DIM = 128
N_CORES = 8
BLK = 128
SEGW = 32  # nodes per segment (aligned dst window)
NSEG = BLK // SEGW  # 4 segments per block
BLOCKS_PER_CORE = 49
NODES_PER_CORE = BLOCKS_PER_CORE * BLK  # 6272
N_PAD = NODES_PER_CORE * N_CORES  # 50176
NBS = BLOCKS_PER_CORE * NSEG  # bins per core (196)
NBINS = N_CORES * NBS  # 1568
TAU = 1.0 / np.sqrt(DIM)


def _balance_bins(deg):
    """Assign each (padded) node to one of NBINS bins of exactly SEGW nodes,
    minimizing the max per-bin degree sum.  Snake dealing + local repair."""
    order = np.argsort(-deg, kind="stable")
    assign = np.empty(N_PAD, np.int64)
    for r in range(SEGW):
        idx = order[r * NBINS : (r + 1) * NBINS]
        bins = np.arange(NBINS) if r % 2 == 0 else np.arange(NBINS)[::-1]
        assign[idx] = bins
    sums = np.zeros(NBINS, np.int64)
    np.add.at(sums, assign, deg)
    nodes_by_bin = [list(np.where(assign == b)[0]) for b in range(NBINS)]
    for _ in range(4000):
        hi = int(np.argmax(sums))
        lo = int(np.argmin(sums))
        gap = int(sums[hi] - sums[lo])
        if gap <= 2:
            break
        want = gap // 2
        da = deg[nodes_by_bin[hi]]
        db = deg[nodes_by_bin[lo]]
        diff = da[:, None] - db[None, :]  # swap gain matrix
        err = np.abs(diff - want)
        err[diff <= 0] = 1 << 30
        ia, ib = np.unravel_index(np.argmin(err), err.shape)
        if diff[ia, ib] <= 0:
            break
        a = nodes_by_bin[hi][ia]
        b = nodes_by_bin[lo][ib]
        nodes_by_bin[hi][ia] = b
        nodes_by_bin[lo][ib] = a
        d = int(deg[a] - deg[b])
        sums[hi] -= d
        sums[lo] += d
        assign[a] = lo
        assign[b] = hi
    return assign


def _prepare(z, Wq, bq, Wk, bk, Wv, bv, src, dst):
    """Host-side sharding: node relabeling, per-core edge grouping into
    (block,segment) chunks, and the source-feature streams."""
    z = np.asarray(z, np.float32)
    src = np.asarray(src, np.int64)
    dst = np.asarray(dst, np.int64)
    assert not np.any(np.asarray(bq)) and not np.any(np.asarray(bk)) and not np.any(
        np.asarray(bv)
    ), "v4 kernel assumes zero biases"

    zT16 = np.zeros((DIM, N_PAD + 1), np.float16)
    zT16[:, :N_NODES] = z.T.astype(np.float16)
    z16 = np.zeros((N_PAD + 1, DIM), np.float16)
    z16[:N_NODES] = z.astype(np.float16)

    deg = np.bincount(dst, minlength=N_PAD)
    assign = _balance_bins(deg)  # node -> bin
    order = np.argsort(assign, kind="stable")
    pos = np.empty(N_PAD, np.int64)
    pos[order] = np.arange(N_PAD) - np.repeat(np.arange(NBINS) * SEGW, SEGW)
    bin_nodes = order.reshape(NBINS, SEGW)  # nodes of bin b in local order

    core_of = assign // NBS
    t_of = assign % NBS

    ecore = core_of[dst]
    eslot = t_of[dst]
    cnts = np.zeros((N_CORES, NBS), np.int64)
    per_core = []
    for c in range(N_CORES):
        sel = ecore == c
        es = src[sel]
        et = eslot[sel]
        ep = pos[dst[sel]]
        o = np.argsort(et, kind="stable")
        es, ep = es[o], ep[o]
        np.add.at(cnts[c], et, 1)
        per_core.append((es, ep))

    Cseg = np.maximum((-(-cnts // BLK)).max(axis=0), 1)  # [196]
    S = int(Cseg.sum())
    offs = np.concatenate([[0], np.cumsum(Cseg)]).astype(int)

    WqT = (np.asarray(Wq, np.float32).T * TAU).astype(np.float16).copy()
    WkT = np.ascontiguousarray(np.asarray(Wk, np.float32).T).astype(np.float16)
    Wv16 = np.asarray(Wv, np.float32).astype(np.float16)
    iota32 = np.broadcast_to(
        np.arange(SEGW, dtype=np.float32), (BLK, SEGW)
    ).copy()  # [e, n] value = n (within segment)
    # fold matrices: F_s[32k+i, 32s+i] = 1 -- collapse per-chunk denominator
    # partial sums (partition 32k+i) onto block node row 32s+i
    fold = np.zeros((BLK, NSEG * BLK), np.float16)
    for s_ in range(NSEG):
        for k_ in range(NSEG):
            for i_ in range(SEGW):
                fold[k_ * SEGW + i_, s_ * BLK + s_ * SEGW + i_] = 1.0

    in_maps = []
    for c in range(N_CORES):
        es, ep = per_core[c]
        col = np.full(S * BLK, N_PAD, np.int64)  # pad -> zero feature row/col
        adj = np.full(S * BLK, -1.0, np.float32)  # pad -> matches no node
        cstart = np.concatenate([[0], np.cumsum(cnts[c])]).astype(int)
        for t in range(NBS):
            n = int(cnts[c][t])
            p0 = int(cstart[t])
            base = int(offs[t]) * BLK
            col[base : base + n] = es[p0 : p0 + n]
            adj[base : base + n] = ep[p0 : p0 + n].astype(np.float32)
        ze = np.ascontiguousarray(zT16[:, col])  # [128, S*128] fp16
        zet = np.ascontiguousarray(
            z16[col].reshape(S, BLK, DIM).transpose(1, 0, 2).reshape(BLK, S * DIM)
        )  # [128(e), S*128] fp16
        dstadj = np.ascontiguousarray(
            adj.reshape(S, BLK).T.astype(np.float32)
        )  # [128(e), S]
        loc_nodes = bin_nodes[c * NBS : (c + 1) * NBS].reshape(-1)
        zq = np.ascontiguousarray(zT16[:, loc_nodes])
        in_maps.append(
            dict(ze=ze, zet=zet, dstadj=dstadj, zq=zq, WqT=WqT, WkT=WkT, Wv=Wv16,
                 iota=iota32, fold=fold)
        )
    consts = dict(Cseg=[int(x) for x in Cseg], S=S)
    return in_maps, consts, bin_nodes


def _build(consts):
    import concourse.bacc as bacc
    import concourse.mybir as mybir
    import concourse.tile as tile
    from concourse.bass import AP

    dt = mybir.dt
    Alu = mybir.AluOpType
    Act = mybir.ActivationFunctionType

    Cseg = consts["Cseg"]
    S = consts["S"]
    offs = np.concatenate([[0], np.cumsum(Cseg)]).astype(int)
    cblk = [int(sum(Cseg[b * NSEG : (b + 1) * NSEG])) for b in range(BLOCKS_PER_CORE)]
    boffs = np.concatenate([[0], np.cumsum(cblk)]).astype(int)
    Cmax = max(Cseg)

    nc = bacc.Bacc("TRN2", target_bir_lowering=False, debug=False, num_devices=N_CORES)

    ze = nc.declare_dram_parameter("ze", [128, S * BLK], dt.float16, isOutput=False)
    zet = nc.declare_dram_parameter("zet", [128, S * BLK], dt.float16, isOutput=False)
    dstadj = nc.declare_dram_parameter("dstadj", [128, S], dt.float32, isOutput=False)
    zq = nc.declare_dram_parameter("zq", [128, NODES_PER_CORE], dt.float16, isOutput=False)
    WqT = nc.declare_dram_parameter("WqT", [128, 128], dt.float16, isOutput=False)
    WkT = nc.declare_dram_parameter("WkT", [128, 128], dt.float16, isOutput=False)
    Wv = nc.declare_dram_parameter("Wv", [128, 128], dt.float16, isOutput=False)
    iota = nc.declare_dram_parameter("iota", [128, SEGW], dt.float32, isOutput=False)
    fold = nc.declare_dram_parameter("fold", [128, NSEG * BLK], dt.float16, isOutput=False)
    h = nc.declare_dram_parameter("h", [NODES_PER_CORE, DIM], dt.float16, isOutput=True)

    with tile.TileContext(nc) as tc:
        with tc.tile_pool(name="const", bufs=1) as constp:
            wqt_sb = constp.tile([128, 128], dt.float16)
            wkt_sb = constp.tile([128, 128], dt.float16)
            wv_sb = constp.tile([128, 128], dt.float16)
            iota_sb = constp.tile([128, SEGW], dt.float32)
            adj_sb = constp.tile([128, S], dt.float32)
            fold_sb = constp.tile([128, NSEG * BLK], dt.float16)
            ones_sb = constp.tile([128, 1], dt.float16)
            nc.sync.dma_start(wqt_sb[:], WqT[:])
            nc.sync.dma_start(wkt_sb[:], WkT[:])
            nc.sync.dma_start(wv_sb[:], Wv[:])
            nc.sync.dma_start(iota_sb[:], iota[:])
            nc.sync.dma_start(adj_sb[:], dstadj[:])
            nc.sync.dma_start(fold_sb[:], fold[:])
            nc.vector.memset(ones_sb[:], 1.0)

            # x[j, i] = (tau Wq Wk^T)[j, i]; per block qk = x^T z_own
            x_sb = constp.tile([128, 128], dt.float16)

            G = 4  # blocks per DMA group
            gsizes = []
            while sum(gsizes) < BLOCKS_PER_CORE:
                gsizes.append(min(G, BLOCKS_PER_CORE - sum(gsizes)))
            gstarts = [0]
            for g in gsizes:
                gstarts.append(gstarts[-1] + g)
            Gmax = max(
                sum(cblk[gstarts[i] : gstarts[i + 1]])
                for i in range(len(gsizes))
            )
            zq_all = constp.tile([128, NODES_PER_CORE], dt.float16)
            with (
                tc.tile_pool(name="zep", bufs=4) as zep,
                tc.tile_pool(name="zetp", bufs=4) as zetp,
                tc.tile_pool(name="qkp", bufs=4) as qkp,
                tc.tile_pool(name="mp", bufs=3) as mp,
                tc.tile_pool(name="pep", bufs=3) as pep,
                tc.tile_pool(name="ptp", bufs=4) as ptp,
                tc.tile_pool(name="usb", bufs=2) as usbp,
                tc.tile_pool(name="dqp", bufs=2) as dqp,
                tc.tile_pool(name="recp", bufs=2) as recp,
                tc.tile_pool(name="hp", bufs=2) as hp,
                tc.tile_pool(name="stps", bufs=2, space="PSUM") as stps,
                tc.tile_pool(name="ups", bufs=3, space="PSUM") as ups,
                tc.tile_pool(name="bkps", bufs=3, space="PSUM") as bkps,
            ):
                blk_state = {}

                def open_block(b):
                    bk = bkps.tile([128, 512], dt.float32, tag="bk")
                    nc.tensor.matmul(
                        bk[:, 0:128], lhsT=x_sb[:],
                        rhs=zq_all[:, b * 128 : (b + 1) * 128], start=True, stop=True
                    )
                    qk_sb = qkp.tile([128, 128], dt.float16, tag="qksb")
                    nc.scalar.copy(qk_sb[:], bk[:, 0:128])
                    u_ps = ups.tile([128, 128], dt.float32, tag="u")
                    blk_state[b] = dict(qk=qk_sb, u=u_ps, bk=bk)

                def close_block(b):
                    st = blk_state.pop(b)
                    bk = st["bk"]
                    u_sb = usbp.tile([128, 128], dt.float16, tag="usb")
                    nc.vector.tensor_copy(u_sb[:], st["u"][:])
                    nc.tensor.matmul(
                        bk[:, 128:256], lhsT=u_sb[:], rhs=wv_sb[:],
                        start=True, stop=True,
                    )
                    # fold per-chunk denominator partials onto node rows
                    dq_sb = dqp.tile([128, NSEG], dt.float16, tag="dq")
                    nc.vector.tensor_copy(dq_sb[:], bk[:, 384 : 384 + NSEG])
                    for s_ in range(NSEG):
                        nc.tensor.matmul(
                            bk[:, 390:391],
                            lhsT=fold_sb[:, s_ * BLK : (s_ + 1) * BLK],
                            rhs=dq_sb[:, s_ : s_ + 1],
                            start=(s_ == 0), stop=(s_ == NSEG - 1),
                            skip_group_check=True,
                        )
                    rec = recp.tile([128, 2], dt.float32, tag="rec")
                    nc.vector.tensor_scalar(
                        out=rec[:, 0:1], in0=bk[:, 390:391], scalar1=1e-12,
                        scalar2=None, op0=Alu.add,
                    )
                    nc.vector.reciprocal(rec[:, 1:2], rec[:, 0:1])
                    ht = hp.tile([128, 128], dt.float16, tag="h")
                    nc.scalar.activation(
                        ht[:], bk[:, 128:256], Act.Copy, scale=rec[:, 1:2]
                    )
                    nc.sync.dma_start(h[b * 128 : (b + 1) * 128, :], ht[:])

                def emit_front(ent):
                    # MM1 logits + mask + exp + mask-multiply for one slot
                    b, seg, t, Cc, sA, s0, zeg, ztg = ent["key"]
                    st = blk_state[b]
                    stt = stps.tile([128, Cmax * SEGW], dt.float32, tag="st")
                    pexp = pep.tile([128, Cmax * SEGW], dt.float16, tag="pe")
                    m4 = mp.tile([128, Cmax * SEGW], dt.float16, tag="m")
                    pt = ptp.tile([128, Cmax * SEGW], dt.float16, tag="pt")
                    w = Cc * SEGW
                    for k in range(Cc):
                        loc = sA + k - s0
                        nc.tensor.matmul(
                            stt[:, k * SEGW : (k + 1) * SEGW],
                            lhsT=zeg[:, loc * BLK : (loc + 1) * BLK],
                            rhs=st["qk"][:, seg * SEGW : (seg + 1) * SEGW],
                            start=True, stop=True,
                        )
                    ia = iota_sb[:]
                    in0 = AP(ia.tensor, ia.offset, [ia.ap[0], [0, Cc], ia.ap[1]])
                    aa = adj_sb[:, sA : sA + Cc]
                    in1 = AP(aa.tensor, aa.offset, [aa.ap[0], aa.ap[1], [0, SEGW]])
                    nc.vector.tensor_tensor(
                        out=m4[:, 0:w], in0=in0, in1=in1, op=Alu.is_equal
                    )
                    nc.scalar.activation(pexp[:, 0:w], stt[:, 0:w], Act.Exp)
                    nc.vector.tensor_mul(pt[:, 0:w], pexp[:, 0:w], m4[:, 0:w])
                    ent["pt"] = pt

                def emit_back(ent):
                    # aggregation U += zet^T P and per-segment denominator
                    b, seg, t, Cc, sA, s0, zeg, ztg = ent["key"]
                    st = blk_state[b]
                    pt = ent["pt"]
                    w = Cc * SEGW
                    for k in range(Cc):
                        loc = sA + k - s0
                        nc.tensor.matmul(
                            st["u"][:, seg * SEGW : (seg + 1) * SEGW],
                            lhsT=ztg[:, loc * BLK : (loc + 1) * BLK],
                            rhs=pt[:, k * SEGW : (k + 1) * SEGW],
                            start=(k == 0), stop=(k == Cc - 1),
                            skip_group_check=True,
                        )
                    # denominator partials: one matmul over up to 128 P columns
                    for w0 in range(0, w, 128):
                        w1 = min(w0 + 128, w)
                        nc.tensor.matmul(
                            st["bk"][0 : w1 - w0, 384 + seg : 385 + seg],
                            lhsT=pt[:, w0:w1],
                            rhs=ones_sb[:],
                            start=(w0 == 0), stop=(w1 == w),
                            skip_group_check=True,
                        )
                    if seg == NSEG - 1:
                        close_block(b)

                # flat slot schedule across DMA groups, with lookahead
                # block opening so qk is ready before a block's first MM1
                slots = []
                for gi in range(len(gsizes)):
                    bl = list(range(gstarts[gi], gstarts[gi + 1]))
                    s0 = int(boffs[bl[0]])
                    nchunk = sum(cblk[b] for b in bl)
                    slots.append(("dma", bl[0], s0, nchunk))
                    for b in bl:
                        for seg in range(NSEG):
                            slots.append(("slot", b, seg, s0))
                pending = []
                SKEW = 3
                OPEN_AHEAD = 5
                cur = {}
                slot_items = [x for x in slots if x[0] == "slot"]
                slot_idx = 0
                for item in slots:
                    if item[0] == "dma":
                        _, b0, s0, nchunk = item
                        zeg = zep.tile([128, Gmax * BLK], dt.float16, tag="ze")
                        nc.sync.dma_start(
                            zeg[:, : nchunk * BLK],
                            ze[:, s0 * BLK : (s0 + nchunk) * BLK],
                        )
                        ztg = zetp.tile([128, Gmax * BLK], dt.float16, tag="zet")
                        nc.sync.dma_start(
                            ztg[:, : nchunk * BLK],
                            zet[:, s0 * BLK : (s0 + nchunk) * BLK],
                        )
                        cur["ze"], cur["zet"] = zeg, ztg
                        if "warmed" not in cur:
                            cur["warmed"] = True
                            nc.sync.dma_start(zq_all[:], zq[:])
                            # PE warm-up (~5us) overlapping the first stream DMA
                            for i in range(48):
                                wps = stps.tile(
                                    [128, Cmax * SEGW], dt.float32, tag="st"
                                )
                                nc.tensor.matmul(
                                    wps[:, 0:128], lhsT=wqt_sb[:], rhs=wkt_sb[:],
                                    start=True, stop=True,
                                )
                                if i == 47:
                                    nc.scalar.copy(x_sb[:], wps[:, 0:128])
                        continue
                    _, b, seg, s0 = item
                    # open this block and a few ahead
                    for ahead in range(OPEN_AHEAD):
                        j = slot_idx + ahead
                        if j < len(slot_items):
                            bb = slot_items[j][1]
                            if bb not in blk_state and bb not in (
                                e["key"][0] for e in pending
                            ):
                                if len(blk_state) < 3:
                                    open_block(bb)
                    if b not in blk_state:
                        open_block(b)
                    t = b * NSEG + seg
                    ent = dict(
                        key=(b, seg, t, Cseg[t], int(offs[t]), s0, cur["ze"], cur["zet"])
                    )
                    emit_front(ent)
                    pending.append(ent)
                    if len(pending) > SKEW:
                        emit_back(pending.pop(0))
                    slot_idx += 1
                while pending:
                    emit_back(pending.pop(0))

    nc.compile()
    return nc


def _install_ntff_hook():
    """The agent image's antenv lacks axon_hooks; recreate it and register
    the ctypes NTFF profile hook the boot would have installed."""
    import types

    if "antenv.axon_hooks" not in sys.modules:
        import antenv

        m = types.ModuleType("antenv.axon_hooks")
        m._hook = None
        m.set_axon_ntff_profile_hook = lambda h, _m=m: setattr(_m, "_hook", h)
        m.get_axon_ntff_profile_hook = lambda _m=m: _m._hook
        sys.modules["antenv.axon_hooks"] = m
        antenv.axon_hooks = m
    from antenv import axon_hooks

    if axon_hooks.get_axon_ntff_profile_hook() is None:
        from trn_agent_boot.trn_boot import _ntff_profile_via_ctypes

        hook = _ntff_profile_via_ctypes("/opt/axon/libaxon_pjrt.so")
        if hook is not None:
            axon_hooks.set_axon_ntff_profile_hook(hook)


def run(inputs, trace=False):
    """Returns (h [50000,128] float32, exec_time_ns or None)."""
    from concourse.bass_utils import run_bass_kernel_spmd

    if trace:
        try:
            _install_ntff_hook()
        except Exception as e:  # profiling is best-effort
            print(f"ntff hook install failed: {e}", file=sys.stderr)

    in_maps, consts, bin_nodes = _prepare(**inputs)
    nc = _build(consts)
    res = run_bass_kernel_spmd(
        nc,
        [dict(m) for m in in_maps],
        list(range(N_CORES)),
        trace=trace,
    )
    hl = np.concatenate([r["h"] for r in res.results], axis=0)  # [N_PAD] local order
    hh = np.empty((N_PAD, DIM), np.float32)
    hh[bin_nodes.reshape(-1)] = hl.astype(np.float32)
    return np.ascontiguousarray(hh[:N_NODES]), res.exec_time_ns


def kernel(**inputs) -> np.ndarray:
    hh, _ = run(inputs, trace=False)
    return hh
